# revision 1
# baseline (speedup 1.0000x reference)
"""HSIC loss kernel for Trainium2, 8-core block-row sharded.

hsic = sum(center(Kx) * center(Ky).T) / (n-1)^2 with
Kx[i,j] = exp(x_i.x_j - ||x_i||^2), Ky[j,i] = exp(y_j.y_i - ||y_j||^2)
(the reference's asymmetric "self-RBF" broadcasting).

Using trace identities (H idempotent), with A=Kx, B=Ky:
  T = sum_ij Ac[i,j]*Bc[j,i] = S_AB - (csA.rsB)/n - (rsA.csB)/n + S_A*S_B/n^2
where S_AB = sum_ij A[i,j]B[j,i], csA = colsums(A), rsA = rowsums(A),
rsB = rowsums(B), csB = colsums(B). Each core owns a 512-row slab of
Ex[i,j] = A[i,j] and Eyt[i,j] = B[j,i] and emits partials; the host sums
the 8 partials and applies the final formula.
"""

import sys

sys.path.insert(0, "/opt/trn_rl_repo")

import numpy as np

P = 128
N = 4096
D = 1024
NCORES = 8
SLAB = N // NCORES        # 512 rows per core
MT = SLAB // P            # 4 m-tiles per slab
CHUNK = 512
NCH = N // CHUNK          # 8 column chunks
KT = D // P               # 8 k-tiles
NTILE = MT * NCH          # 32 out-tiles per matrix per core

_compiled = {}


def _build_program():
    import concourse.bacc as bacc
    import concourse.mybir as mybir
    import concourse.tile as tile

    f32 = mybir.dt.float32
    f16 = mybir.dt.float16
    bf16 = mybir.dt.bfloat16
    Exp = mybir.ActivationFunctionType.Exp
    mult = mybir.AluOpType.mult
    add = mybir.AluOpType.add

    nc = bacc.Bacc("TRN2", target_bir_lowering=False, debug=False,
                   num_devices=NCORES)

    xt = nc.dram_tensor("xt", [D, N], f16, kind="ExternalInput")
    yt = nc.dram_tensor("yt", [D, N], f16, kind="ExternalInput")
    xs = nc.dram_tensor("xs", [D, SLAB], f16, kind="ExternalInput")
    ys = nc.dram_tensor("ys", [D, SLAB], f16, kind="ExternalInput")
    sqxn = nc.dram_tensor("sqxn", [P, MT], f32, kind="ExternalInput")
    ybias = nc.dram_tensor("ybias", [P, N], f16, kind="ExternalInput")

    o_csa = nc.dram_tensor("o_csa", [1, N], f32, kind="ExternalOutput")
    o_rsb = nc.dram_tensor("o_rsb", [1, N], f32, kind="ExternalOutput")
    o_rsa = nc.dram_tensor("o_rsa", [P, NTILE], f32, kind="ExternalOutput")
    o_csb = nc.dram_tensor("o_csb", [P, NTILE], f32, kind="ExternalOutput")
    o_p = nc.dram_tensor("o_p", [P, 1], f32, kind="ExternalOutput")

    with tile.TileContext(nc) as tc:
        with (
            tc.tile_pool(name="big", bufs=1) as big,
            tc.tile_pool(name="work", bufs=4) as work,
            tc.tile_pool(name="small", bufs=2) as small,
            tc.tile_pool(name="psum", bufs=4, space="PSUM") as pp,
            tc.tile_pool(name="psacc", bufs=2, space="PSUM") as ppacc,
        ):
            xs_sb = big.tile([P, KT, SLAB], f16, tag="xs")
            xt_sb = big.tile([P, KT, N], f16, tag="xt")
            ys_sb = big.tile([P, KT, SLAB], f16, tag="ys")
            yt_sb = big.tile([P, KT, N], f16, tag="yt")
            yb_sb = big.tile([P, N], f16, tag="yb")
            sqx_sb = big.tile([P, MT], f32, tag="sq")
            ones16 = big.tile([P, P], f16, tag="ones16")
            onesb = big.tile([P, 1], bf16, tag="onesb")
            ex_sb = big.tile([P, NTILE, CHUNK], bf16, tag="ex")
            rsa_sb = big.tile([P, NTILE], f32, tag="rsa")
            csb_sb = big.tile([P, NTILE], f32, tag="csb")
            pacc_sb = big.tile([P, CHUNK], bf16, tag="pacc")
            p_sb = big.tile([P, 1], f32, tag="pout")

            # input loads (x side first: x-phase can start earliest)
            for k in range(KT):
                nc.sync.dma_start(xs_sb[:, k], xs[k * P:(k + 1) * P, :])
            for k in range(KT):
                nc.sync.dma_start(xt_sb[:, k], xt[k * P:(k + 1) * P, :])
            nc.sync.dma_start(sqx_sb[:], sqxn[:])
            for k in range(KT):
                nc.sync.dma_start(ys_sb[:, k], ys[k * P:(k + 1) * P, :])
            for k in range(KT):
                nc.sync.dma_start(yt_sb[:, k], yt[k * P:(k + 1) * P, :])
            nc.sync.dma_start(yb_sb[:], ybias[:])
            nc.any.memset(ones16[:], 1.0)
            nc.any.memset(onesb[:], 1.0)
            nc.any.memset(pacc_sb[:], 0.0)

            # ---- x phase: Ex tiles (retained), rsA, csA ----
            for c in range(NCH):
                csa_ps = ppacc.tile([1, CHUNK], f32, tag="acc")
                for m in range(MT):
                    t = c * MT + m
                    ps = pp.tile([P, CHUNK], f32, tag="mm")
                    for k in range(KT):
                        nc.tensor.matmul(
                            ps,
                            xs_sb[:, k, m * P:(m + 1) * P],
                            xt_sb[:, k, c * CHUNK:(c + 1) * CHUNK],
                            start=(k == 0),
                            stop=(k == KT - 1),
                        )
                    nc.scalar.activation(
                        ex_sb[:, t], ps, Exp,
                        bias=sqx_sb[:, m:m + 1],
                        accum_out=rsa_sb[:, t:t + 1],
                    )
                    nc.tensor.matmul(
                        csa_ps, onesb, ex_sb[:, t],
                        start=(m == 0), stop=(m == MT - 1),
                    )
                csa_row = small.tile([1, CHUNK], f32, tag="accrow")
                nc.any.tensor_copy(csa_row[:], csa_ps)
                nc.sync.dma_start(o_csa[:, c * CHUNK:(c + 1) * CHUNK], csa_row[:])

            # ---- y phase: Eyt tiles, csB, rsB, product accumulation ----
            for c in range(NCH):
                rsb_ps = ppacc.tile([1, CHUNK], f32, tag="acc")
                for m in range(MT):
                    t = c * MT + m
                    ps = pp.tile([P, CHUNK], f32, tag="mm")
                    for k in range(KT):
                        nc.tensor.matmul(
                            ps,
                            ys_sb[:, k, m * P:(m + 1) * P],
                            yt_sb[:, k, c * CHUNK:(c + 1) * CHUNK],
                            start=(k == 0),
                            stop=False,
                        )
                    # bias rows: adds -sqy[j] (split hi+lo) to every row
                    nc.tensor.matmul(
                        ps, ones16, yb_sb[:, c * CHUNK:(c + 1) * CHUNK],
                        start=False, stop=True,
                    )
                    eyt = work.tile([P, CHUNK], bf16, tag="eyt")
                    nc.scalar.activation(
                        eyt[:], ps, Exp,
                        accum_out=csb_sb[:, t:t + 1],
                    )
                    nc.tensor.matmul(
                        rsb_ps, onesb, eyt[:],
                        start=(m == 0), stop=(m == MT - 1),
                    )
                    scr = work.tile([P, CHUNK], bf16, tag="scr")
                    nc.vector.tensor_tensor(scr[:], ex_sb[:, t], eyt[:], mult)
                    nc.vector.tensor_tensor(pacc_sb[:], pacc_sb[:], scr[:], add)
                rsb_row = small.tile([1, CHUNK], f32, tag="accrow")
                nc.any.tensor_copy(rsb_row[:], rsb_ps)
                nc.sync.dma_start(o_rsb[:, c * CHUNK:(c + 1) * CHUNK], rsb_row[:])

            nc.vector.reduce_sum(p_sb[:], pacc_sb[:], axis=mybir.AxisListType.X)
            nc.sync.dma_start(o_rsa[:], rsa_sb[:])
            nc.sync.dma_start(o_csb[:], csb_sb[:])
            nc.sync.dma_start(o_p[:], p_sb[:])

    nc.compile()
    return nc


def _get_program():
    if "nc" not in _compiled:
        _compiled["nc"] = _build_program()
    return _compiled["nc"]


def prepare_in_maps(x: np.ndarray, y: np.ndarray):
    """Host-side layout prep + sharding: returns per-core input maps."""
    xt = np.ascontiguousarray(x.T.astype(np.float16))   # [D, N]
    yt = np.ascontiguousarray(y.T.astype(np.float16))

    # row norms consistent with the fp16 data the device actually dots
    sqx = (xt.astype(np.float32) ** 2).sum(axis=0)      # [N]
    sqy = (yt.astype(np.float32) ** 2).sum(axis=0)

    # y-side bias, split so fp16 rows carry full f32 precision:
    s_hi = (-sqy).astype(np.float16)
    s_lo = ((-sqy).astype(np.float32) - s_hi.astype(np.float32)).astype(np.float16)
    ybias = np.zeros((P, N), dtype=np.float16)
    ybias[0] = s_hi
    ybias[1] = s_lo

    in_maps = []
    for d in range(NCORES):
        sl = slice(d * SLAB, (d + 1) * SLAB)
        in_maps.append({
            "xt": xt,
            "yt": yt,
            "xs": np.ascontiguousarray(xt[:, sl]),
            "ys": np.ascontiguousarray(yt[:, sl]),
            "sqxn": np.ascontiguousarray((-sqx[sl]).reshape(MT, P).T),
            "ybias": ybias,
        })
    return in_maps


def combine_results(results):
    """Sum per-core partials and apply the final HSIC formula (host)."""
    n = float(N)
    csa = np.zeros(N, dtype=np.float64)
    rsb = np.zeros(N, dtype=np.float64)
    s_ab = 0.0
    dot_rc = 0.0
    for r in results:
        csa += r["o_csa"].astype(np.float64).ravel()
        rsb += r["o_rsb"].astype(np.float64).ravel()
        s_ab += float(r["o_p"].astype(np.float64).sum())
        rsa = r["o_rsa"].astype(np.float64)
        csb = r["o_csb"].astype(np.float64)
        dot_rc += float((rsa * csb).sum())
    s_a = float(csa.sum())
    s_b = float(rsb.sum())
    t = s_ab - float(csa @ rsb) / n - dot_rc / n + s_a * s_b / (n * n)
    return np.float32(t / ((n - 1.0) ** 2))


def kernel(x: np.ndarray, y: np.ndarray) -> np.ndarray:
    from concourse.bass_utils import run_bass_kernel_spmd

    nc = _get_program()
    in_maps = prepare_in_maps(np.asarray(x), np.asarray(y))
    res = run_bass_kernel_spmd(nc, in_maps, core_ids=list(range(NCORES)))
    return combine_results(res.results)



# revision 17
# speedup vs baseline: 2.3408x; 2.3408x over previous
"""HSIC loss kernel for Trainium2, 8-core block-row sharded, fp8 DoubleRow.

hsic = sum(center(Kx) * center(Ky).T) / (n-1)^2 with
Kx[i,j] = exp(x_i.x_j - ||x_i||^2), Ky[j,i] = exp(y_j.y_i - ||y_j||^2)
(the reference's asymmetric "self-RBF" broadcasting).

Using trace identities (H idempotent), with A=Kx, B=Ky:
  T = S_AB - (csA.rsB)/n - (rsA.csB)/n + S_A*S_B/n^2
where S_AB = sum_ij A[i,j]B[j,i], csA = colsums(A), rsA = rowsums(A),
rsB = rowsums(B), csB = colsums(B).

Each core owns a 512-row slab of Ex[i,j] = A[i,j] and Eyt[i,j] = B[j,i]:
 - inputs quantized to fp8e4 on host; row norms recomputed from the
   quantized values so the diagonal exponent stays ~0.
 - main dots via fp8 DoubleRow matmuls (2 k-tiles per instruction).
 - y-side per-column bias -||y_j||^2 folded in as a 5th DoubleRow pass
   (ones-rows stationary x fp8 cascade rows moving).
 - exp on the activation engine over 3-bank PSUM groups; accum_out
   yields row sums (rsA / csB partials).
 - partition-axis sums (csA / rsB / S_AB) via near-free PE matmuls with
   the data tile stationary and a [P,1] ones moving operand, chained
   start=False accumulation into a dedicated PSUM bank.
Host sums the 8 partial sets and applies the final formula in f64.
"""

import sys

sys.path.insert(0, "/opt/trn_rl_repo")

import numpy as np
import ml_dtypes

P = 128
N = 4096
D = 1024
NCORES = 8
SLAB = N // NCORES        # 512 rows per core
MT = SLAB // P            # 4 m-tiles per slab
CHUNK = 512
NCH = N // CHUNK          # 8 column chunks
KT = D // P               # 8 k-tiles
KP = KT // 2              # 4 DoubleRow k-pairs
NBIAS = 8                 # fp8 cascade rows for -||y||^2

# activation groups per m-tile: (pool id, chunk list); pool 0 is reused
# for the third group after its first activation drains.
GROUPS = [(0, (0, 1, 2)), (1, (3, 4, 5)), (0, (6, 7))]
NG = len(GROUPS)

_compiled = {}


def _build_program():
    import concourse.bacc as bacc
    import concourse.mybir as mybir
    import concourse.tile as tile

    f32 = mybir.dt.float32
    f8 = mybir.dt.float8e4
    bf16 = mybir.dt.bfloat16
    Exp = mybir.ActivationFunctionType.Exp
    mult = mybir.AluOpType.mult
    DR = mybir.MatmulPerfMode.DoubleRow

    nc = bacc.Bacc("TRN2", target_bir_lowering=False, debug=False,
                   num_devices=NCORES)

    xt = nc.dram_tensor("xt", [P, KT, N], f8, kind="ExternalInput")
    yt = nc.dram_tensor("yt", [P, KT, N], f8, kind="ExternalInput")
    sqxn = nc.dram_tensor("sqxn", [P, MT], f32, kind="ExternalInput")
    ybias = nc.dram_tensor("ybias", [P, 2, N], f8, kind="ExternalInput")
    onesr = nc.dram_tensor("onesr", [P, 2, P], f8, kind="ExternalInput")

    o_rsa = nc.dram_tensor("o_rsa", [P, MT * NG], f32, kind="ExternalOutput")
    o_csb = nc.dram_tensor("o_csb", [P, MT * NG], f32, kind="ExternalOutput")
    o_acc = nc.dram_tensor("o_acc", [P, 65], f32, kind="ExternalOutput")

    with tile.TileContext(nc) as tc:
        with (
            tc.tile_pool(name="big", bufs=1) as big,
            tc.tile_pool(name="eywork", bufs=3) as eywork,
            tc.tile_pool(name="scwork", bufs=4) as scwork,
            tc.tile_pool(name="pa", bufs=1, space="PSUM") as pa,
            tc.tile_pool(name="pb", bufs=1, space="PSUM") as pb,
            tc.tile_pool(name="pacc", bufs=1, space="PSUM") as pacc,
            tc.tile_pool(name="pd", bufs=1, space="PSUM") as pd,
        ):
            xt_sb = big.tile([P, KT, N], f8, tag="xt")
            yt_sb = big.tile([P, KT, N], f8, tag="yt")
            sqx_sb = big.tile([P, MT], f32, tag="sq")
            yb_sb = big.tile([P, 2, N], f8, tag="yb")
            onesr_sb = big.tile([P, 2, P], f8, tag="onesr")
            ones1 = big.tile([P, 1], bf16, tag="ones1")
            ex_sb = big.tile([P, MT, NCH, CHUNK], bf16, tag="ex")
            rsa_sb = big.tile([P, MT * NG], f32, tag="rsa")
            csb_sb = big.tile([P, MT * NG], f32, tag="csb")
            accout_sb = big.tile([P, 65], f32, tag="accout")

            acc_ps = pacc.tile([P, 65], f32, tag="acc")
            pools = (pa, pb)

            # input loads, ordered so compute can start early
            for c in range(3):
                nc.sync.dma_start(xt_sb[:, :, c * CHUNK:(c + 1) * CHUNK],
                                  xt[:, :, c * CHUNK:(c + 1) * CHUNK])
            nc.sync.dma_start(sqx_sb[:], sqxn[:])
            for c in range(3, NCH):
                nc.sync.dma_start(xt_sb[:, :, c * CHUNK:(c + 1) * CHUNK],
                                  xt[:, :, c * CHUNK:(c + 1) * CHUNK])
            nc.sync.dma_start(yb_sb[:], ybias[:])
            nc.sync.dma_start(onesr_sb[:], onesr[:])
            for c in range(NCH):
                nc.sync.dma_start(yt_sb[:, :, c * CHUNK:(c + 1) * CHUNK],
                                  yt[:, :, c * CHUNK:(c + 1) * CHUNK])
            nc.any.memset(ones1[:], 1.0)
            nc.vector.memset(acc_ps[:], 0.0)

            # ---- x phase ----
            # columns are pre-rotated per core: device column j' is global
            # column (512*device + j') % N, so the slab sits at chunks 0-3
            # of... chunk 0's q-block m holds the m-block diagonal.
            for m in range(MT):
                sl = slice(m * P, (m + 1) * P)
                for gi, (pi, chunks) in enumerate(GROUPS):
                    gl = len(chunks)
                    ps = pools[pi].tile([P, 3, CHUNK], f32, tag=f"ps{pi}")
                    for ci, c in enumerate(chunks):
                        cs = slice(c * CHUNK, (c + 1) * CHUNK)
                        for kp in range(KP):
                            nc.tensor.matmul(
                                ps[:, ci],
                                xt_sb[:, 2 * kp:2 * kp + 2, sl],
                                xt_sb[:, 2 * kp:2 * kp + 2, cs],
                                start=(kp == 0), stop=(kp == KP - 1),
                                perf_mode=DR,
                            )
                    g = m * NG + gi
                    nc.scalar.activation(
                        ex_sb[:, m, chunks[0]:chunks[0] + gl],
                        ps[:, 0:gl], Exp,
                        bias=sqx_sb[:, m:m + 1],
                        accum_out=rsa_sb[:, g:g + 1],
                    )
                    if gi == 0:
                        # DoubleRow truncates (~-6e-5 rel of the sum), which
                        # matters only where exp() is not ~0: the diagonal
                        # block. Recompute it with exact plain-fp8 matmuls.
                        psd = pd.tile([P, P], f32, tag="psd")
                        for k in range(KT):
                            nc.tensor.matmul(
                                psd, xt_sb[:, k, sl], xt_sb[:, k, sl],
                                start=(k == 0), stop=(k == KT - 1),
                            )
                        nc.scalar.activation(
                            ex_sb[:, m, 0, sl], psd, Exp,
                            bias=sqx_sb[:, m:m + 1],
                        )
                    # csA partials: partition-axis sums of each 128-col block
                    for c in chunks:
                        for q in range(4):
                            t = c * 4 + q
                            nc.tensor.matmul(
                                acc_ps[:, t:t + 1],
                                ex_sb[:, m, c, q * P:(q + 1) * P],
                                ones1[:],
                                start=False, stop=(m == MT - 1),
                                skip_group_check=True,
                            )

            # ---- y phase ----
            for m in range(MT):
                sl = slice(m * P, (m + 1) * P)
                for gi, (pi, chunks) in enumerate(GROUPS):
                    gl = len(chunks)
                    ps = pools[pi].tile([P, 3, CHUNK], f32, tag=f"ps{pi}")
                    for ci, c in enumerate(chunks):
                        cs = slice(c * CHUNK, (c + 1) * CHUNK)
                        for kp in range(KP):
                            nc.tensor.matmul(
                                ps[:, ci],
                                yt_sb[:, 2 * kp:2 * kp + 2, sl],
                                yt_sb[:, 2 * kp:2 * kp + 2, cs],
                                start=(kp == 0), stop=False,
                                perf_mode=DR,
                            )
                        nc.tensor.matmul(
                            ps[:, ci], onesr_sb[:], yb_sb[:, :, cs],
                            start=False, stop=True, perf_mode=DR,
                        )
                    g = m * NG + gi
                    eyt = eywork.tile([P, 3, CHUNK], bf16, tag="eyt")
                    nc.scalar.activation(
                        eyt[:, 0:gl], ps[:, 0:gl], Exp,
                        accum_out=csb_sb[:, g:g + 1],
                    )
                    if gi == 0:
                        # exact diagonal block (see x phase)
                        psd = pd.tile([P, P], f32, tag="psd")
                        for k in range(KT):
                            nc.tensor.matmul(
                                psd, yt_sb[:, k, sl], yt_sb[:, k, sl],
                                start=(k == 0), stop=False,
                            )
                        nc.tensor.matmul(
                            psd, onesr_sb[:, 0], yb_sb[:, 0, sl],
                            start=False, stop=True,
                        )
                        nc.scalar.activation(eyt[:, 0, sl], psd, Exp)
                    last = (m == MT - 1)
                    for ci, c in enumerate(chunks):
                        scr = scwork.tile([P, CHUNK], bf16, tag="scr")
                        nc.vector.tensor_tensor(
                            scr[:], ex_sb[:, m, c], eyt[:, ci], mult)
                        for q in range(4):
                            t = 32 + c * 4 + q
                            nc.tensor.matmul(
                                acc_ps[:, t:t + 1],
                                eyt[:, ci, q * P:(q + 1) * P], ones1[:],
                                start=False, stop=last,
                                skip_group_check=True,
                            )
                        for q in range(4):
                            nc.tensor.matmul(
                                acc_ps[:, 64:65],
                                scr[:, q * P:(q + 1) * P], ones1[:],
                                start=False,
                                stop=(last and c == 7 and q == 3),
                                skip_group_check=True,
                            )

            nc.any.tensor_copy(accout_sb[:], acc_ps[:])
            nc.sync.dma_start(o_acc[:], accout_sb[:])
            nc.sync.dma_start(o_rsa[:], rsa_sb[:])
            nc.sync.dma_start(o_csb[:], csb_sb[:])

    nc.compile()
    return nc


def _get_program():
    if "nc" not in _compiled:
        _compiled["nc"] = _build_program()
    return _compiled["nc"]


def _f8cast(a):
    return a.astype(ml_dtypes.float8_e4m3)


def prepare_in_maps(x: np.ndarray, y: np.ndarray):
    """Host-side fp8 quantize + layout prep; same maps for every core
    (the program selects its slab via device_id)."""
    x8 = _f8cast(np.asarray(x, dtype=np.float32))
    y8 = _f8cast(np.asarray(y, dtype=np.float32))

    # norms of the quantized values the device actually dots
    sqx = (x8.astype(np.float32) ** 2).sum(axis=1)     # [N]
    sqy = (y8.astype(np.float32) ** 2).sum(axis=1)

    # [P, KT, N]: xtr[p, k, j] = x8[j, k*128+p]
    xtr = np.ascontiguousarray(
        x8.T.reshape(KT, P, N).transpose(1, 0, 2))
    ytr = np.ascontiguousarray(
        y8.T.reshape(KT, P, N).transpose(1, 0, 2))

    # fp8 cascade rows summing to -sqy (error ~1e-3)
    r = (-sqy).astype(np.float64)
    ybias = np.zeros((P, 2, N), dtype=ml_dtypes.float8_e4m3)
    for i in range(NBIAS):
        h = _f8cast(np.clip(r, -240.0, 240.0).astype(np.float32))
        ybias[i, 0, :] = h
        r = r - h.astype(np.float64)

    onesr = np.zeros((P, 2, P), dtype=ml_dtypes.float8_e4m3)
    onesr[0:NBIAS, 0, :] = 1.0

    in_maps = []
    for d in range(NCORES):
        sl = slice(d * SLAB, (d + 1) * SLAB)
        sh = -d * SLAB
        # rotate columns so this core's slab sits at device columns 0-511
        in_maps.append({
            "xt": np.ascontiguousarray(np.roll(xtr, sh, axis=2)),
            "yt": np.ascontiguousarray(np.roll(ytr, sh, axis=2)),
            "sqxn": np.ascontiguousarray((-sqx[sl]).reshape(MT, P).T),
            "ybias": np.ascontiguousarray(np.roll(ybias, sh, axis=2)),
            "onesr": onesr,
        })
    return in_maps


def combine_results(results):
    """Sum per-core partials and apply the final HSIC formula (host, f64)."""
    n = float(N)
    csa = np.zeros(N, dtype=np.float64)
    rsb = np.zeros(N, dtype=np.float64)
    rsa = np.zeros(N, dtype=np.float64)
    csb = np.zeros(N, dtype=np.float64)
    s_ab = 0.0
    for d, r in enumerate(results):
        acc = r["o_acc"].astype(np.float64)              # [P, 65]
        # col t=c*4+q, partition p -> device column j' = c*512 + q*128 + p;
        # global column j = (d*512 + j') % N  (columns were pre-rotated)
        csa += np.roll(
            acc[:, 0:32].reshape(P, NCH, 4).transpose(1, 2, 0).ravel(),
            d * SLAB)
        rsb += np.roll(
            acc[:, 32:64].reshape(P, NCH, 4).transpose(1, 2, 0).ravel(),
            d * SLAB)
        s_ab += acc[:, 64].sum()
        sl = slice(d * SLAB, (d + 1) * SLAB)
        # [P, MT*NG] -> sum groups -> row i = m*128 + p within the slab
        rsa[sl] = (r["o_rsa"].astype(np.float64)
                   .reshape(P, MT, NG).sum(axis=2).T.ravel())
        csb[sl] = (r["o_csb"].astype(np.float64)
                   .reshape(P, MT, NG).sum(axis=2).T.ravel())
    s_a = csa.sum()
    s_b = rsb.sum()
    t = s_ab - (csa @ rsb) / n - (rsa @ csb) / n + s_a * s_b / (n * n)
    return np.float32(t / ((n - 1.0) ** 2))


def kernel(x: np.ndarray, y: np.ndarray) -> np.ndarray:
    from concourse.bass_utils import run_bass_kernel_spmd

    nc = _get_program()
    in_maps = prepare_in_maps(np.asarray(x), np.asarray(y))
    res = run_bass_kernel_spmd(nc, in_maps, core_ids=list(range(NCORES)))
    return combine_results(res.results)
